# revision 10
# baseline (speedup 1.0000x reference)
"""Embedding-lookup kernel for Trainium2 (8 NeuronCores, SPMD batch-parallel).

Problem (hardcoded): B=4096, L=50, V=100000, D=64.
  - 4 "hist" tables [V, D]: gather [B, L, D], mean over L -> [B, D]
  - 4 "cat" tables  [V, D]: gather [B, 1, D]            -> [B, D]
  - output: concat -> [B, 8*D] = [4096, 512] float32

Architecture (per core, 512 batch rows, 104448 lookups):
  * All 8 tables stacked host-side into ONE bf16 table [800000, 128]:
    cols 0:64 = bf16(rows), cols 64:128 unused pad.  The 256B row stride
    satisfies the SWDGE gather's source-stride constraint while each
    gathered element is only 128B (elem_size=64 bf16) - the ucode allows
    any elem size; only the Python wrapper asserts %256 (bypassed here).
  * Value space split into 13 windows of 61600 rows.  int16 gather indices
    are CENTER-relative (the Q7 ucode sign-extends; reach +/-32767), so a
    window is ~2x the naive 32000-row limit -> half the windows/calls.
  * One dma_gather per window (~8k descriptors), round-robin across the 4
    SWDGE queues.  Trailing-negative indices are swapped away per window
    (the ucode strips trailing negatives).
  * Lookups sorted by (window, dest).  Slot j of a window lands at SBUF
    [j%128, j//128].  Each 128-slot column is scatter-accumulated into a
    PSUM acc [128, 32*64] via a selection matmul (is_equal mask of the
    host-precomputed group-biased dest stream against iota).  Mask
    generation is batched 16 columns per DVE op.
  * SPMD: single instruction stream; window sizes padded to the cross-core
    max; per-column group lists are unions across cores.
  * Epilogue: acc -> out tiles with 1/L scale on the hist half.
"""

import numpy as np

B, L, V, D = 4096, 50, 100000, 64
NCORES = 8
BPC = B // NCORES
P = 128
NTAB = 8
VSTACK = NTAB * V            # 800000
# Table-aligned windows: each stacked table occupies [t*100000, (t+1)*100000);
# 2 windows of 50000 rows per table so no window straddles a table boundary
# (a straddling window would give some dest blocks one-sided index signs and
# break the per-call trailing-negative guard).
WSPAN = 50000
NW = 16
WCTR = [w * WSPAN + 25000 for w in range(NW)]


def _win_of(vals):
    return vals // WSPAN
NDEST = 4096
NG = NDEST // P              # 32 psum groups
NLOOK = BPC * (4 * L + 4)    # 104448
MB = 32                      # mask batch (columns per DVE is_equal op)

_cache = {}


def _raw_dma_gather(gp, out_ap, in_ap, idxs_ap, num_idxs, elem_size, elem_step,
                    queue_num):
    """dma_gather without the elem_size%256 restriction (the ucode only
    requires the source stride to be a 256B multiple)."""
    from concourse import mybir
    stride_bytes = elem_step * mybir.dt.size(in_ap.dtype)
    assert stride_bytes % 256 == 0
    _in_ap = gp.lower_ap_dma(in_ap, for_custom_bir_dma=True)
    _idxs_ap = gp.lower_ap(idxs_ap)
    _out_ap = gp.lower_ap(out_ap)
    return gp.add_instruction(
        mybir.InstDMAGatherAnt(
            name=gp.bass.get_next_instruction_name(),
            ins=[*_in_ap, _idxs_ap, gp.lower_val_access(gp.to_reg(num_idxs))],
            outs=[_out_ap],
            transpose=False,
            num_idxs=num_idxs,
            elem_size=elem_size,
            stride_bytes_256=stride_bytes // 256,
            gen_mode=0,
            single_packet=False,
            queue_num=queue_num,
        )
    )


def _prep(inputs):
    import ml_dtypes
    hist = [np.asarray(inputs[f"hist{i}"], dtype=np.int64) for i in range(4)]
    cat = [np.asarray(inputs[f"cat{i}"], dtype=np.int64) for i in range(4)]
    w_hist = [np.asarray(inputs[f"W_hist{i}"], dtype=np.float32) for i in range(4)]
    w_cat = [np.asarray(inputs[f"W_cat{i}"], dtype=np.float32) for i in range(4)]
    tf32 = np.concatenate(w_hist + w_cat, axis=0)          # [800000, 64] f32
    table = np.zeros((VSTACK, 2 * D), ml_dtypes.bfloat16)
    table[:, :D] = tf32.astype(ml_dtypes.bfloat16)

    # per-core (value, dest) streams
    vals = np.empty((NCORES, NLOOK), np.int64)
    dsts = np.empty((NCORES, NLOOK), np.int64)
    for c in range(NCORES):
        b0 = c * BPC
        vparts, dparts = [], []
        for t in range(4):
            vparts.append((hist[t][b0:b0 + BPC] + t * V).ravel())
            dparts.append(np.repeat(np.arange(BPC), L) + t * BPC)
        for t in range(4):
            vparts.append((cat[t][b0:b0 + BPC] + (4 + t) * V).ravel())
            dparts.append(np.arange(BPC) + 2048 + t * BPC)
        vals[c] = np.concatenate(vparts)
        dsts[c] = np.concatenate(dparts)

    win = _win_of(vals)
    order = np.lexsort((dsts, win), axis=-1)
    vals = np.take_along_axis(vals, order, axis=1)
    dsts = np.take_along_axis(dsts, order, axis=1)
    win = np.take_along_axis(win, order, axis=1)

    counts = np.stack([np.bincount(win[c], minlength=NW) for c in range(NCORES)])
    a_w = [int(np.ceil(counts[:, w].max() / P)) for w in range(NW)]
    ncols = sum(a_w)

    idx_stream = np.zeros((NCORES, ncols * P), np.int16)
    dst_stream = np.full((NCORES, ncols * P), -1, np.int32)
    offs = np.concatenate([[0], np.cumsum([a * P for a in a_w])])
    for c in range(NCORES):
        pos = np.concatenate([[0], np.cumsum(counts[c])])
        for w in range(NW):
            n = counts[c, w]
            seg = vals[c, pos[w]:pos[w] + n] - WCTR[w]
            assert n == 0 or (seg.min() >= -32768 and seg.max() < 32768)
            idx_stream[c, offs[w]:offs[w] + n] = seg.astype(np.int16)
            dst_stream[c, offs[w]:offs[w] + n] = dsts[c, pos[w]:pos[w] + n]
            # trailing-negative guard per CALL (<=31 cols each): the ucode
            # strips trailing negative idxs of every dma_gather instruction
            for s0 in range(0, a_w[w], 31):
                s1 = min(s0 + 31, a_w[w])
                lo, hi = offs[w] + s0 * P, offs[w] + s1 * P
                if idx_stream[c, hi - 1] < 0:
                    blk_i = idx_stream[c, lo:hi]
                    blk_d = dst_stream[c, lo:hi]
                    nz = np.flatnonzero(blk_i >= 0)[-1]
                    blk_i[nz], blk_i[-1] = blk_i[-1], blk_i[nz]
                    blk_d[nz], blk_d[-1] = blk_d[-1], blk_d[nz]

    # per-column group unions + bias streams
    dcols = dst_stream.reshape(NCORES, ncols, P)
    entries = []                      # (col, group) in column order
    for cidx in range(ncols):
        g = dcols[:, cidx, :]
        g = g[g >= 0] // P
        for gg in sorted(set(int(x) for x in np.unique(g))):
            entries.append((cidx, gg))
    nbias = len(entries)
    # pad entries to a multiple of MB (dummy bias columns, no matmul)
    nbias_pad = -(-nbias // MB) * MB
    bias = np.full((NCORES, nbias_pad, P), -1.0, np.float32)
    for j, (cidx, gg) in enumerate(entries):
        bias[:, j, :] = dcols[:, cidx, :] - P * gg

    # device layouts
    wrapped = idx_stream.reshape(NCORES, ncols * 8, 16)
    idx_dev = np.tile(np.transpose(wrapped, (0, 2, 1)), (1, 8, 1))
    bias_dev = np.ascontiguousarray(
        np.transpose(bias, (0, 2, 1))).astype(ml_dtypes.bfloat16)
    iota = np.tile(np.arange(P, dtype=np.float32)[None, :],
                   (P, MB)).astype(ml_dtypes.bfloat16)   # [128, MB*128]

    meta = {"a_w": a_w, "entries": entries, "nbias_pad": nbias_pad}
    in_maps = [
        {
            "table": table,
            "gidx": np.ascontiguousarray(idx_dev[c]),
            "bias": bias_dev[c],
            "iota": iota,
        }
        for c in range(NCORES)
    ]
    return meta, in_maps


def _build(meta):
    from concourse import bacc, mybir
    from concourse.tile import TileContext

    a_w = meta["a_w"]
    entries = meta["entries"]
    nbias_pad = meta["nbias_pad"]
    ncols = sum(a_w)

    nc = bacc.Bacc(
        "TRN2",
        target_bir_lowering=False,
        debug=False,
        num_devices=NCORES,
        num_swdge_queues=4,
    )
    table = nc.dram_tensor(
        "table", [VSTACK, 2 * D], mybir.dt.bfloat16, kind="ExternalInput").ap()
    gidx = nc.dram_tensor(
        "gidx", [P, ncols * 8], mybir.dt.int16, kind="ExternalInput").ap()
    bias = nc.dram_tensor(
        "bias", [P, nbias_pad], mybir.dt.bfloat16, kind="ExternalInput").ap()
    iota = nc.dram_tensor(
        "iota", [P, MB * P], mybir.dt.bfloat16, kind="ExternalInput").ap()
    out = nc.dram_tensor(
        "out", [BPC, NTAB * D], mybir.dt.float32, kind="ExternalOutput").ap()

    with TileContext(nc) as tc:
        with (
            tc.tile_pool(name="cst", bufs=1) as cst,
            tc.tile_pool(name="gp", bufs=1) as gp,
            tc.tile_pool(name="mp", bufs=4) as mp,
            tc.tile_pool(name="pp", bufs=1, space="PSUM") as pp,
            tc.tile_pool(name="op", bufs=2) as op,
        ):
            it = cst.tile([P, MB * P], mybir.dt.bfloat16)
            nc.sync.dma_start(out=it[:], in_=iota[:])
            idx_all = cst.tile([P, ncols * 8], mybir.dt.int16)
            bias_all = cst.tile([P, nbias_pad], mybir.dt.bfloat16)
            nc.sync.dma_start(out=bias_all[:], in_=bias[:])
            acc = pp.tile([P, NG * D], mybir.dt.float32, space="PSUM")
            nc.vector.memset(acc[:], 0.0)

            gts = []          # per-window gather tiles
            col0 = 0
            qn = 0
            for w, aw in enumerate(a_w):
                gt = gp.tile([P, aw, D], mybir.dt.bfloat16, tag=f"gt{w}")
                center = WCTR[w]
                # per-window idx slice load (so window w's gathers only wait
                # for their own indices)
                nc.sync.dma_start(
                    out=idx_all[:, col0 * 8:(col0 + aw) * 8],
                    in_=gidx[:, col0 * 8:(col0 + aw) * 8])
                # <=31-col calls: 3968 descs = 249/ring, two calls fit in
                # the 512-desc ring carveout -> 2-deep pipelining per queue
                # while amortizing the ~2.3us per-call Q7 dispatch overhead
                for s0 in range(0, aw, 31):
                    s1 = min(s0 + 31, aw)
                    _raw_dma_gather(
                        nc.gpsimd, gt[:, s0:s1, :],
                        table[center:center + 2, :D],
                        idx_all[:, (col0 + s0) * 8:(col0 + s1) * 8],
                        (s1 - s0) * P, D, 2 * D,
                        queue_num=qn % 4,
                    )
                    qn += 1
                gts.append((gt, col0, aw))
                col0 += aw
            assert col0 == ncols

            def col_tile(cidx):
                for gt, c0, aw in gts:
                    if c0 <= cidx < c0 + aw:
                        return gt[:, cidx - c0, :]
                raise AssertionError(cidx)

            for e0 in range(0, nbias_pad, MB):
                mk = mp.tile([P, MB, P], mybir.dt.bfloat16, tag="mk")
                nc.vector.tensor_tensor(
                    out=mk[:],
                    in0=bias_all[:, e0:e0 + MB].to_broadcast([P, MB, P]),
                    in1=it[:].rearrange("p (m q) -> p m q", m=MB),
                    op=mybir.AluOpType.is_equal,
                )
                for k in range(MB):
                    j = e0 + k
                    if j >= len(entries):
                        break
                    cidx, g = entries[j]
                    nc.tensor.matmul(
                        out=acc[:, g * D:(g + 1) * D],
                        lhsT=mk[:, k, :],
                        rhs=col_tile(cidx),
                        start=False,
                        stop=True,
                    )

            accv = acc[:].rearrange("p (g e) -> p g e", g=NG)
            for bb in range(BPC // P):
                ot = op.tile([P, NTAB * D], mybir.dt.float32, tag="ot")
                otv = ot[:].rearrange("p (t e) -> p t e", t=NTAB)
                nc.scalar.mul(
                    out=otv[:, :4, :], in_=accv[:, bb:16:4, :], mul=1.0 / L)
                nc.vector.tensor_copy(
                    out=otv[:, 4:, :], in_=accv[:, 16 + bb:32:4, :])
                nc.sync.dma_start(out=out[bb * P:(bb + 1) * P, :], in_=ot[:])
    nc.compile()
    return nc


def _run(inputs, **spmd_kwargs):
    meta, in_maps = _prep(inputs)
    key = (tuple(meta["a_w"]), meta["nbias_pad"],
           hash(tuple(meta["entries"])))
    if key not in _cache:
        _cache[key] = _build(meta)
    from concourse.bass_utils import run_bass_kernel_spmd

    res = run_bass_kernel_spmd(
        _cache[key], in_maps, core_ids=list(range(NCORES)), **spmd_kwargs
    )
    outp = np.concatenate(
        [res.results[c]["out"] for c in range(NCORES)], axis=0
    )
    return outp, res


def kernel(**inputs) -> np.ndarray:
    outp, _ = _run(inputs)
    return outp


# revision 11
# speedup vs baseline: 1.1444x; 1.1444x over previous
"""Embedding-lookup kernel for Trainium2 (8 NeuronCores, SPMD batch-parallel).

Problem (hardcoded): B=4096, L=50, V=100000, D=64.
  - 4 "hist" tables [V, D]: gather [B, L, D], mean over L -> [B, D]
  - 4 "cat" tables  [V, D]: gather [B, 1, D]            -> [B, D]
  - output: concat -> [B, 8*D] = [4096, 512] float32

Architecture (per core, 512 batch rows, 104448 lookups):
  * All 8 tables stacked host-side into ONE bf16 table [800000, 128]:
    cols 0:64 = bf16(rows), cols 64:128 unused pad.  The 256B row stride
    satisfies the SWDGE gather's source-stride constraint while each
    gathered element is only 128B (elem_size=64 bf16) - the ucode allows
    any elem size; only the Python wrapper asserts %256 (bypassed here).
  * Value space split into 13 windows of 61600 rows.  int16 gather indices
    are CENTER-relative (the Q7 ucode sign-extends; reach +/-32767), so a
    window is ~2x the naive 32000-row limit -> half the windows/calls.
  * One dma_gather per window (~8k descriptors), round-robin across the 4
    SWDGE queues.  Trailing-negative indices are swapped away per window
    (the ucode strips trailing negatives).
  * Lookups sorted by (window, dest).  Slot j of a window lands at SBUF
    [j%128, j//128].  Each 128-slot column is scatter-accumulated into a
    PSUM acc [128, 32*64] via a selection matmul (is_equal mask of the
    host-precomputed group-biased dest stream against iota).  Mask
    generation is batched 16 columns per DVE op.
  * SPMD: single instruction stream; window sizes padded to the cross-core
    max; per-column group lists are unions across cores.
  * Epilogue: acc -> out tiles with 1/L scale on the hist half.
"""

import numpy as np

B, L, V, D = 4096, 50, 100000, 64
NCORES = 8
BPC = B // NCORES
P = 128
NTAB = 8
VSTACK = NTAB * V            # 800000
# Table-aligned windows: each stacked table occupies [t*100000, (t+1)*100000);
# 2 windows of 50000 rows per table so no window straddles a table boundary
# (a straddling window would give some dest blocks one-sided index signs and
# break the per-call trailing-negative guard).
WSPAN = 50000
NW = 16
WCTR = [w * WSPAN + 25000 for w in range(NW)]


def _win_of(vals):
    return vals // WSPAN
NDEST = 4096
NG = NDEST // P              # 32 psum groups
NLOOK = BPC * (4 * L + 4)    # 104448
MB = 16                      # mask batch (columns per DVE is_equal op)

_cache = {}


def _raw_dma_gather(gp, out_ap, in_ap, idxs_ap, num_idxs, elem_size, elem_step,
                    queue_num):
    """dma_gather without the elem_size%256 restriction (the ucode only
    requires the source stride to be a 256B multiple)."""
    from concourse import mybir
    stride_bytes = elem_step * mybir.dt.size(in_ap.dtype)
    assert stride_bytes % 256 == 0
    _in_ap = gp.lower_ap_dma(in_ap, for_custom_bir_dma=True)
    _idxs_ap = gp.lower_ap(idxs_ap)
    _out_ap = gp.lower_ap(out_ap)
    return gp.add_instruction(
        mybir.InstDMAGatherAnt(
            name=gp.bass.get_next_instruction_name(),
            ins=[*_in_ap, _idxs_ap, gp.lower_val_access(gp.to_reg(num_idxs))],
            outs=[_out_ap],
            transpose=False,
            num_idxs=num_idxs,
            elem_size=elem_size,
            stride_bytes_256=stride_bytes // 256,
            gen_mode=0,
            single_packet=False,
            queue_num=queue_num,
        )
    )


def _prep(inputs):
    import ml_dtypes
    hist = [np.asarray(inputs[f"hist{i}"], dtype=np.int64) for i in range(4)]
    cat = [np.asarray(inputs[f"cat{i}"], dtype=np.int64) for i in range(4)]
    w_hist = [np.asarray(inputs[f"W_hist{i}"], dtype=np.float32) for i in range(4)]
    w_cat = [np.asarray(inputs[f"W_cat{i}"], dtype=np.float32) for i in range(4)]
    tf32 = np.concatenate(w_hist + w_cat, axis=0)          # [800000, 64] f32
    table = np.zeros((VSTACK, 2 * D), ml_dtypes.bfloat16)
    table[:, :D] = tf32.astype(ml_dtypes.bfloat16)

    # per-core (value, dest) streams
    vals = np.empty((NCORES, NLOOK), np.int64)
    dsts = np.empty((NCORES, NLOOK), np.int64)
    for c in range(NCORES):
        b0 = c * BPC
        vparts, dparts = [], []
        for t in range(4):
            vparts.append((hist[t][b0:b0 + BPC] + t * V).ravel())
            dparts.append(np.repeat(np.arange(BPC), L) + t * BPC)
        for t in range(4):
            vparts.append((cat[t][b0:b0 + BPC] + (4 + t) * V).ravel())
            dparts.append(np.arange(BPC) + 2048 + t * BPC)
        vals[c] = np.concatenate(vparts)
        dsts[c] = np.concatenate(dparts)

    win = _win_of(vals)
    order = np.lexsort((dsts, win), axis=-1)
    vals = np.take_along_axis(vals, order, axis=1)
    dsts = np.take_along_axis(dsts, order, axis=1)
    win = np.take_along_axis(win, order, axis=1)

    counts = np.stack([np.bincount(win[c], minlength=NW) for c in range(NCORES)])
    a_w = [int(np.ceil(counts[:, w].max() / P)) for w in range(NW)]
    ncols = sum(a_w)

    idx_stream = np.zeros((NCORES, ncols * P), np.int16)
    dst_stream = np.full((NCORES, ncols * P), -1, np.int32)
    offs = np.concatenate([[0], np.cumsum([a * P for a in a_w])])
    for c in range(NCORES):
        pos = np.concatenate([[0], np.cumsum(counts[c])])
        for w in range(NW):
            n = counts[c, w]
            seg = vals[c, pos[w]:pos[w] + n] - WCTR[w]
            assert n == 0 or (seg.min() >= -32768 and seg.max() < 32768)
            idx_stream[c, offs[w]:offs[w] + n] = seg.astype(np.int16)
            dst_stream[c, offs[w]:offs[w] + n] = dsts[c, pos[w]:pos[w] + n]
            # trailing-negative guard per CALL (<=15 cols each): the ucode
            # strips trailing negative idxs of every dma_gather instruction
            for s0 in range(0, a_w[w], 15):
                s1 = min(s0 + 15, a_w[w])
                lo, hi = offs[w] + s0 * P, offs[w] + s1 * P
                if idx_stream[c, hi - 1] < 0:
                    blk_i = idx_stream[c, lo:hi]
                    blk_d = dst_stream[c, lo:hi]
                    nz = np.flatnonzero(blk_i >= 0)[-1]
                    blk_i[nz], blk_i[-1] = blk_i[-1], blk_i[nz]
                    blk_d[nz], blk_d[-1] = blk_d[-1], blk_d[nz]

    # per-column group unions + bias streams
    dcols = dst_stream.reshape(NCORES, ncols, P)
    entries = []                      # (col, group) in column order
    for cidx in range(ncols):
        g = dcols[:, cidx, :]
        g = g[g >= 0] // P
        for gg in sorted(set(int(x) for x in np.unique(g))):
            entries.append((cidx, gg))
    nbias = len(entries)
    # pad entries to a multiple of MB (dummy bias columns, no matmul)
    nbias_pad = -(-nbias // MB) * MB
    bias = np.full((NCORES, nbias_pad, P), -1.0, np.float32)
    for j, (cidx, gg) in enumerate(entries):
        bias[:, j, :] = dcols[:, cidx, :] - P * gg

    # device layouts
    wrapped = idx_stream.reshape(NCORES, ncols * 8, 16)
    idx_dev = np.tile(np.transpose(wrapped, (0, 2, 1)), (1, 8, 1))
    bias_dev = np.ascontiguousarray(
        np.transpose(bias, (0, 2, 1))).astype(ml_dtypes.bfloat16)
    iota = np.tile(np.arange(P, dtype=np.float32)[None, :],
                   (P, MB)).astype(ml_dtypes.bfloat16)   # [128, MB*128]

    meta = {"a_w": a_w, "entries": entries, "nbias_pad": nbias_pad}
    in_maps = [
        {
            "table": table,
            "gidx": np.ascontiguousarray(idx_dev[c]),
            "bias": bias_dev[c],
            "iota": iota,
        }
        for c in range(NCORES)
    ]
    return meta, in_maps


def _build(meta):
    from concourse import bacc, mybir
    from concourse.tile import TileContext

    a_w = meta["a_w"]
    entries = meta["entries"]
    nbias_pad = meta["nbias_pad"]
    ncols = sum(a_w)

    nc = bacc.Bacc(
        "TRN2",
        target_bir_lowering=False,
        debug=False,
        num_devices=NCORES,
        num_swdge_queues=4,
    )
    table = nc.dram_tensor(
        "table", [VSTACK, 2 * D], mybir.dt.bfloat16, kind="ExternalInput").ap()
    gidx = nc.dram_tensor(
        "gidx", [P, ncols * 8], mybir.dt.int16, kind="ExternalInput").ap()
    bias = nc.dram_tensor(
        "bias", [P, nbias_pad], mybir.dt.bfloat16, kind="ExternalInput").ap()
    iota = nc.dram_tensor(
        "iota", [P, MB * P], mybir.dt.bfloat16, kind="ExternalInput").ap()
    out = nc.dram_tensor(
        "out", [BPC, NTAB * D], mybir.dt.float32, kind="ExternalOutput").ap()

    with TileContext(nc) as tc:
        with (
            tc.tile_pool(name="cst", bufs=1) as cst,
            tc.tile_pool(name="gp", bufs=1) as gp,
            tc.tile_pool(name="mp", bufs=6) as mp,
            tc.tile_pool(name="pp", bufs=1, space="PSUM") as pp,
            tc.tile_pool(name="op", bufs=2) as op,
        ):
            it = cst.tile([P, MB * P], mybir.dt.bfloat16)
            nc.sync.dma_start(out=it[:], in_=iota[:])
            idx_all = cst.tile([P, ncols * 8], mybir.dt.int16)
            bias_all = cst.tile([P, nbias_pad], mybir.dt.bfloat16)
            nc.sync.dma_start(out=bias_all[:], in_=bias[:])
            acc = pp.tile([P, NG * D], mybir.dt.float32, space="PSUM")
            nc.vector.memset(acc[:], 0.0)

            gts = []          # per-window gather tiles
            col0 = 0
            qn = 0
            for w, aw in enumerate(a_w):
                gt = gp.tile([P, aw, D], mybir.dt.bfloat16, tag=f"gt{w}")
                center = WCTR[w]
                # per-window idx slice load (so window w's gathers only wait
                # for their own indices)
                nc.sync.dma_start(
                    out=idx_all[:, col0 * 8:(col0 + aw) * 8],
                    in_=gidx[:, col0 * 8:(col0 + aw) * 8])
                # <=15-col calls: 1920 descs = 121/ring, two calls fit in the
                # 256-desc ring carveout -> 2-deep pipelining per queue while
                # amortizing the ~2.3us per-call Q7 dispatch overhead
                for s0 in range(0, aw, 15):
                    s1 = min(s0 + 15, aw)
                    _raw_dma_gather(
                        nc.gpsimd, gt[:, s0:s1, :],
                        table[center:center + 2, :D],
                        idx_all[:, (col0 + s0) * 8:(col0 + s1) * 8],
                        (s1 - s0) * P, D, 2 * D,
                        queue_num=qn % 4,
                    )
                    qn += 1
                gts.append((gt, col0, aw))
                col0 += aw
            assert col0 == ncols

            def col_tile(cidx):
                for gt, c0, aw in gts:
                    if c0 <= cidx < c0 + aw:
                        return gt[:, cidx - c0, :]
                raise AssertionError(cidx)

            for e0 in range(0, nbias_pad, MB):
                mk = mp.tile([P, MB, P], mybir.dt.bfloat16, tag="mk")
                nc.vector.tensor_tensor(
                    out=mk[:],
                    in0=bias_all[:, e0:e0 + MB].to_broadcast([P, MB, P]),
                    in1=it[:].rearrange("p (m q) -> p m q", m=MB),
                    op=mybir.AluOpType.is_equal,
                )
                for k in range(MB):
                    j = e0 + k
                    if j >= len(entries):
                        break
                    cidx, g = entries[j]
                    nc.tensor.matmul(
                        out=acc[:, g * D:(g + 1) * D],
                        lhsT=mk[:, k, :],
                        rhs=col_tile(cidx),
                        start=False,
                        stop=True,
                    )

            accv = acc[:].rearrange("p (g e) -> p g e", g=NG)
            for bb in range(BPC // P):
                ot = op.tile([P, NTAB * D], mybir.dt.float32, tag="ot")
                otv = ot[:].rearrange("p (t e) -> p t e", t=NTAB)
                nc.scalar.mul(
                    out=otv[:, :4, :], in_=accv[:, bb:16:4, :], mul=1.0 / L)
                nc.vector.tensor_copy(
                    out=otv[:, 4:, :], in_=accv[:, 16 + bb:32:4, :])
                nc.sync.dma_start(out=out[bb * P:(bb + 1) * P, :], in_=ot[:])
    nc.compile()
    return nc


def _run(inputs, **spmd_kwargs):
    meta, in_maps = _prep(inputs)
    key = (tuple(meta["a_w"]), meta["nbias_pad"],
           hash(tuple(meta["entries"])))
    if key not in _cache:
        _cache[key] = _build(meta)
    from concourse.bass_utils import run_bass_kernel_spmd

    res = run_bass_kernel_spmd(
        _cache[key], in_maps, core_ids=list(range(NCORES)), **spmd_kwargs
    )
    outp = np.concatenate(
        [res.results[c]["out"] for c in range(NCORES)], axis=0
    )
    return outp, res


def kernel(**inputs) -> np.ndarray:
    outp, _ = _run(inputs)
    return outp


# revision 14
# speedup vs baseline: 1.1465x; 1.0019x over previous
"""Embedding-lookup kernel for Trainium2 (8 NeuronCores, SPMD batch-parallel).

Problem (hardcoded): B=4096, L=50, V=100000, D=64.
  - 4 "hist" tables [V, D]: gather [B, L, D], mean over L -> [B, D]
  - 4 "cat" tables  [V, D]: gather [B, 1, D]            -> [B, D]
  - output: concat -> [B, 8*D] = [4096, 512] float32

Architecture (per core, 512 batch rows, 104448 lookups):
  * All 8 tables stacked host-side into ONE bf16 table [800000, 128]:
    cols 0:64 = bf16(rows), cols 64:128 unused pad.  The 256B row stride
    satisfies the SWDGE gather's source-stride constraint while each
    gathered element is only 128B (elem_size=64 bf16) - the ucode allows
    any elem size; only the Python wrapper asserts %256 (bypassed here).
  * Value space split into 13 windows of 61600 rows.  int16 gather indices
    are CENTER-relative (the Q7 ucode sign-extends; reach +/-32767), so a
    window is ~2x the naive 32000-row limit -> half the windows/calls.
  * One dma_gather per window (~8k descriptors), round-robin across the 4
    SWDGE queues.  Trailing-negative indices are swapped away per window
    (the ucode strips trailing negatives).
  * Lookups sorted by (window, dest).  Slot j of a window lands at SBUF
    [j%128, j//128].  Each 128-slot column is scatter-accumulated into a
    PSUM acc [128, 32*64] via a selection matmul (is_equal mask of the
    host-precomputed group-biased dest stream against iota).  Mask
    generation is batched 16 columns per DVE op.
  * SPMD: single instruction stream; window sizes padded to the cross-core
    max; per-column group lists are unions across cores.
  * Epilogue: acc -> out tiles with 1/L scale on the hist half.
"""

import numpy as np

B, L, V, D = 4096, 50, 100000, 64
NCORES = 8
BPC = B // NCORES
P = 128
NTAB = 8
VSTACK = NTAB * V            # 800000
# Table-aligned windows: each stacked table occupies [t*100000, (t+1)*100000);
# 2 windows of 50000 rows per table so no window straddles a table boundary
# (a straddling window would give some dest blocks one-sided index signs and
# break the per-call trailing-negative guard).
WSPAN = 50000
NW = 16
WCTR = [w * WSPAN + 25000 for w in range(NW)]


def _win_of(vals):
    return vals // WSPAN
NDEST = 4096
NG = NDEST // P              # 32 psum groups
NLOOK = BPC * (4 * L + 4)    # 104448
MB = 16                      # mask batch (columns per DVE is_equal op)

_cache = {}


def _raw_dma_gather(gp, out_ap, in_ap, idxs_ap, num_idxs, elem_size, elem_step,
                    queue_num):
    """dma_gather without the elem_size%256 restriction (the ucode only
    requires the source stride to be a 256B multiple)."""
    from concourse import mybir
    stride_bytes = elem_step * mybir.dt.size(in_ap.dtype)
    assert stride_bytes % 256 == 0
    _in_ap = gp.lower_ap_dma(in_ap, for_custom_bir_dma=True)
    _idxs_ap = gp.lower_ap(idxs_ap)
    _out_ap = gp.lower_ap(out_ap)
    return gp.add_instruction(
        mybir.InstDMAGatherAnt(
            name=gp.bass.get_next_instruction_name(),
            ins=[*_in_ap, _idxs_ap, gp.lower_val_access(gp.to_reg(num_idxs))],
            outs=[_out_ap],
            transpose=False,
            num_idxs=num_idxs,
            elem_size=elem_size,
            stride_bytes_256=stride_bytes // 256,
            gen_mode=0,
            single_packet=True,
            queue_num=queue_num,
        )
    )


def _prep(inputs):
    import ml_dtypes
    hist = [np.asarray(inputs[f"hist{i}"], dtype=np.int64) for i in range(4)]
    cat = [np.asarray(inputs[f"cat{i}"], dtype=np.int64) for i in range(4)]
    w_hist = [np.asarray(inputs[f"W_hist{i}"], dtype=np.float32) for i in range(4)]
    w_cat = [np.asarray(inputs[f"W_cat{i}"], dtype=np.float32) for i in range(4)]
    tf32 = np.concatenate(w_hist + w_cat, axis=0)          # [800000, 64] f32
    table = np.zeros((VSTACK, 2 * D), ml_dtypes.bfloat16)
    table[:, :D] = tf32.astype(ml_dtypes.bfloat16)

    # per-core (value, dest) streams
    vals = np.empty((NCORES, NLOOK), np.int64)
    dsts = np.empty((NCORES, NLOOK), np.int64)
    for c in range(NCORES):
        b0 = c * BPC
        vparts, dparts = [], []
        for t in range(4):
            vparts.append((hist[t][b0:b0 + BPC] + t * V).ravel())
            dparts.append(np.repeat(np.arange(BPC), L) + t * BPC)
        for t in range(4):
            vparts.append((cat[t][b0:b0 + BPC] + (4 + t) * V).ravel())
            dparts.append(np.arange(BPC) + 2048 + t * BPC)
        vals[c] = np.concatenate(vparts)
        dsts[c] = np.concatenate(dparts)

    win = _win_of(vals)
    order = np.lexsort((dsts, win), axis=-1)
    vals = np.take_along_axis(vals, order, axis=1)
    dsts = np.take_along_axis(dsts, order, axis=1)
    win = np.take_along_axis(win, order, axis=1)

    counts = np.stack([np.bincount(win[c], minlength=NW) for c in range(NCORES)])
    a_w = [int(np.ceil(counts[:, w].max() / P)) for w in range(NW)]
    ncols = sum(a_w)

    idx_stream = np.zeros((NCORES, ncols * P), np.int16)
    dst_stream = np.full((NCORES, ncols * P), -1, np.int32)
    offs = np.concatenate([[0], np.cumsum([a * P for a in a_w])])
    for c in range(NCORES):
        pos = np.concatenate([[0], np.cumsum(counts[c])])
        for w in range(NW):
            n = counts[c, w]
            seg = vals[c, pos[w]:pos[w] + n] - WCTR[w]
            assert n == 0 or (seg.min() >= -32768 and seg.max() < 32768)
            idx_stream[c, offs[w]:offs[w] + n] = seg.astype(np.int16)
            dst_stream[c, offs[w]:offs[w] + n] = dsts[c, pos[w]:pos[w] + n]
            # trailing-negative guard per CALL (<=15 cols each): the ucode
            # strips trailing negative idxs of every dma_gather instruction
            for s0 in range(0, a_w[w], 15):
                s1 = min(s0 + 15, a_w[w])
                lo, hi = offs[w] + s0 * P, offs[w] + s1 * P
                if idx_stream[c, hi - 1] < 0:
                    blk_i = idx_stream[c, lo:hi]
                    blk_d = dst_stream[c, lo:hi]
                    nz = np.flatnonzero(blk_i >= 0)[-1]
                    blk_i[nz], blk_i[-1] = blk_i[-1], blk_i[nz]
                    blk_d[nz], blk_d[-1] = blk_d[-1], blk_d[nz]

    # per-column group unions + bias streams
    dcols = dst_stream.reshape(NCORES, ncols, P)
    entries = []                      # (col, group) in column order
    for cidx in range(ncols):
        g = dcols[:, cidx, :]
        g = g[g >= 0] // P
        for gg in sorted(set(int(x) for x in np.unique(g))):
            entries.append((cidx, gg))
    nbias = len(entries)
    # pad entries to a multiple of MB (dummy bias columns, no matmul)
    nbias_pad = -(-nbias // MB) * MB
    bias = np.full((NCORES, nbias_pad, P), -1.0, np.float32)
    for j, (cidx, gg) in enumerate(entries):
        bias[:, j, :] = dcols[:, cidx, :] - P * gg

    # device layouts
    wrapped = idx_stream.reshape(NCORES, ncols * 8, 16)
    idx_dev = np.tile(np.transpose(wrapped, (0, 2, 1)), (1, 8, 1))
    bias_dev = np.ascontiguousarray(
        np.transpose(bias, (0, 2, 1))).astype(ml_dtypes.bfloat16)
    iota = np.tile(np.arange(P, dtype=np.float32)[None, :],
                   (P, MB)).astype(ml_dtypes.bfloat16)   # [128, MB*128]

    meta = {"a_w": a_w, "entries": entries, "nbias_pad": nbias_pad}
    in_maps = [
        {
            "table": table,
            "gidx": np.ascontiguousarray(idx_dev[c]),
            "bias": bias_dev[c],
            "iota": iota,
        }
        for c in range(NCORES)
    ]
    return meta, in_maps


def _build(meta):
    from concourse import bacc, mybir
    from concourse.tile import TileContext

    a_w = meta["a_w"]
    entries = meta["entries"]
    nbias_pad = meta["nbias_pad"]
    ncols = sum(a_w)

    nc = bacc.Bacc(
        "TRN2",
        target_bir_lowering=False,
        debug=False,
        num_devices=NCORES,
        num_swdge_queues=4,
    )
    table = nc.dram_tensor(
        "table", [VSTACK, 2 * D], mybir.dt.bfloat16, kind="ExternalInput").ap()
    gidx = nc.dram_tensor(
        "gidx", [P, ncols * 8], mybir.dt.int16, kind="ExternalInput").ap()
    bias = nc.dram_tensor(
        "bias", [P, nbias_pad], mybir.dt.bfloat16, kind="ExternalInput").ap()
    iota = nc.dram_tensor(
        "iota", [P, MB * P], mybir.dt.bfloat16, kind="ExternalInput").ap()
    out = nc.dram_tensor(
        "out", [BPC, NTAB * D], mybir.dt.float32, kind="ExternalOutput").ap()

    with TileContext(nc) as tc:
        with (
            tc.tile_pool(name="cst", bufs=1) as cst,
            tc.tile_pool(name="gp", bufs=1) as gp,
            tc.tile_pool(name="mp", bufs=6) as mp,
            tc.tile_pool(name="pp", bufs=1, space="PSUM") as pp,
            tc.tile_pool(name="op", bufs=2) as op,
        ):
            it = cst.tile([P, MB * P], mybir.dt.bfloat16)
            nc.sync.dma_start(out=it[:], in_=iota[:])
            idx_all = cst.tile([P, ncols * 8], mybir.dt.int16)
            bias_all = cst.tile([P, nbias_pad], mybir.dt.bfloat16)
            nc.sync.dma_start(out=bias_all[:], in_=bias[:])
            acc = pp.tile([P, NG * D], mybir.dt.float32, space="PSUM")
            nc.vector.memset(acc[:], 0.0)

            # cat windows (8..15, tiny) first: their idx slices land fast,
            # letting the Pool pipeline start early; hist windows follow.
            worder = list(range(8, NW)) + list(range(8))
            offs_d = np.concatenate([[0], np.cumsum(a_w)]).astype(int)
            gtiles = {}
            qn = 0
            for w in worder:
                aw = a_w[w]
                col0 = int(offs_d[w])
                gt = gp.tile([P, aw, D], mybir.dt.bfloat16, tag=f"gt{w}")
                center = WCTR[w]
                # per-window idx slice load (so window w's gathers only wait
                # for their own indices)
                nc.sync.dma_start(
                    out=idx_all[:, col0 * 8:(col0 + aw) * 8],
                    in_=gidx[:, col0 * 8:(col0 + aw) * 8])
                # <=15-col calls: 1920 descs = 121/ring, two calls fit in the
                # 256-desc ring carveout -> 2-deep pipelining per queue while
                # amortizing the ~2.3us per-call Q7 dispatch overhead
                for s0 in range(0, aw, 15):
                    s1 = min(s0 + 15, aw)
                    _raw_dma_gather(
                        nc.gpsimd, gt[:, s0:s1, :],
                        table[center:center + 2, :D],
                        idx_all[:, (col0 + s0) * 8:(col0 + s1) * 8],
                        (s1 - s0) * P, D, 2 * D,
                        queue_num=qn % 4,
                    )
                    qn += 1
                gtiles[w] = (gt, col0, aw)
            gts = [gtiles[w] for w in range(NW)]

            def col_tile(cidx):
                for gt, c0, aw in gts:
                    if c0 <= cidx < c0 + aw:
                        return gt[:, cidx - c0, :]
                raise AssertionError(cidx)

            for e0 in range(0, nbias_pad, MB):
                mk = mp.tile([P, MB, P], mybir.dt.bfloat16, tag="mk")
                nc.vector.tensor_tensor(
                    out=mk[:],
                    in0=bias_all[:, e0:e0 + MB].to_broadcast([P, MB, P]),
                    in1=it[:].rearrange("p (m q) -> p m q", m=MB),
                    op=mybir.AluOpType.is_equal,
                )
                for k in range(MB):
                    j = e0 + k
                    if j >= len(entries):
                        break
                    cidx, g = entries[j]
                    nc.tensor.matmul(
                        out=acc[:, g * D:(g + 1) * D],
                        lhsT=mk[:, k, :],
                        rhs=col_tile(cidx),
                        start=False,
                        stop=True,
                    )

            accv = acc[:].rearrange("p (g e) -> p g e", g=NG)
            for bb in range(BPC // P):
                ot = op.tile([P, NTAB * D], mybir.dt.float32, tag="ot")
                otv = ot[:].rearrange("p (t e) -> p t e", t=NTAB)
                nc.scalar.mul(
                    out=otv[:, :4, :], in_=accv[:, bb:16:4, :], mul=1.0 / L)
                nc.vector.tensor_copy(
                    out=otv[:, 4:, :], in_=accv[:, 16 + bb:32:4, :])
                nc.sync.dma_start(out=out[bb * P:(bb + 1) * P, :], in_=ot[:])
    nc.compile()
    return nc


def _run(inputs, **spmd_kwargs):
    meta, in_maps = _prep(inputs)
    key = (tuple(meta["a_w"]), meta["nbias_pad"],
           hash(tuple(meta["entries"])))
    if key not in _cache:
        _cache[key] = _build(meta)
    from concourse.bass_utils import run_bass_kernel_spmd

    res = run_bass_kernel_spmd(
        _cache[key], in_maps, core_ids=list(range(NCORES)), **spmd_kwargs
    )
    outp = np.concatenate(
        [res.results[c]["out"] for c in range(NCORES)], axis=0
    )
    return outp, res


def kernel(**inputs) -> np.ndarray:
    outp, _ = _run(inputs)
    return outp
